# revision 22
# baseline (speedup 1.0000x reference)
"""Causal self-attention (B=4, T=2048, C=2048, H=16) on 8 trn2 NeuronCores.

Sharding: tensor-parallel over heads (2 heads/core). Each core computes the
QKV projection for its head shard (q,k produced transposed for the scores
matmul, v in normal layout), applies rope fused into the PSUM->SBUF drain
(4 DVE/GpSimd ops per tile), and runs causal attention without
max-subtraction (scores are O(5), exp is fp32-safe). All matmuls are bf16.

Softmax denominator: computed on the PE as accumulating ones-column
matmuls, but over DVE-compacted PAIRS of exp tiles (bf16 pair-adds halve
the den matmul count); diagonal score tiles are column-sliced so only the
un-masked tq range is computed (scores, exp, mask, den and attn@v all
operate on the slice).

Output tokens are round-robin sharded: core c projects tokens
[256c, 256(c+1)) of every batch. Attention output Y^T re-shards via one
small AllToAll per (batch, head) — 8 collectives total. The program is
software-pipelined so every softmax-reciprocal, A2A transfer and the
output projection is covered by independent PE work: batch b's h=0
finalize runs under h=1's score chains, h=1's finalize runs under batch
b+1's first QKV chunk, proj(b) runs between batch b+1's QKV and
attention, and the last batch's finalize+A2A hides under proj(2). The
host reassembles the [B, 256, C] per-core chunks into the full output.
"""

import os
import sys

os.environ.setdefault("JAX_PLATFORMS", "axon")

import numpy as np

B, T, C = 4, 2048, 2048
H = 16
HD = 128
N_CORES = 8
HL = H // N_CORES  # heads per core = 2
CL = HL * HD  # per-core head columns = 256
TQ = 512  # Tq chunk for scores
NJ = T // TQ  # 4 q-chunks
NKT = T // 128  # 16 tiles of 128 along T
KC = C // 128  # 16 k-tiles along C
TOK = T // N_CORES  # output tokens per core per batch = 256


def _install_ntff_shim():
    """The agent image's antenv lacks axon_hooks; provide it so
    run_bass_kernel_spmd(trace=True) can reach the NTFF profiler."""
    import types, contextlib, ctypes

    try:
        from antenv.axon_hooks import get_axon_ntff_profile_hook  # noqa

        return
    except ImportError:
        pass

    so_path = "/opt/axon/libaxon_pjrt.so"
    try:
        lib = ctypes.CDLL(so_path)
    except OSError:
        lib = None
    if lib is None or not hasattr(lib, "axon_start_nrt_profile"):
        hook = None
    else:
        lib.axon_start_nrt_profile.argtypes = [
            ctypes.POINTER(ctypes.c_int64),
            ctypes.c_size_t,
        ]
        lib.axon_start_nrt_profile.restype = ctypes.c_int64
        lib.axon_stop_nrt_profile.argtypes = [ctypes.c_char_p]
        lib.axon_stop_nrt_profile.restype = ctypes.c_int64

        @contextlib.contextmanager
        def hook(output_dir, device_ids):
            import jax

            jax.devices()
            if device_ids:
                ids = (ctypes.c_int64 * len(device_ids))(*device_ids)
                rc = lib.axon_start_nrt_profile(ids, len(device_ids))
            else:
                rc = lib.axon_start_nrt_profile(None, 0)
            if rc != 0:
                raise RuntimeError(f"axon_start_nrt_profile rc={rc}")
            try:
                yield
            finally:
                n = lib.axon_stop_nrt_profile(str(output_dir).encode())
                if n <= 0:
                    print(f"ntff profile: rc={n} (no files) dir={output_dir}")

    import antenv

    mod = types.ModuleType("antenv.axon_hooks")
    _state = {"hook": hook}
    mod.set_axon_ntff_profile_hook = lambda h: _state.__setitem__("hook", h)
    mod.get_axon_ntff_profile_hook = lambda: _state["hook"]
    sys.modules["antenv.axon_hooks"] = mod
    antenv.axon_hooks = mod


def build_program():
    import concourse.bass as bass
    import concourse.mybir as mybir
    import concourse.tile as tile
    from concourse import bacc
    from contextlib import ExitStack

    f32 = mybir.dt.float32
    f32r = mybir.dt.float32r
    bf16 = mybir.dt.bfloat16
    mdt = bf16
    Exp = mybir.ActivationFunctionType.Exp

    nc = bacc.Bacc("TRN2", target_bir_lowering=False, debug=False, num_devices=N_CORES)

    xT = nc.dram_tensor("xT", [B, C, T], mdt, kind="ExternalInput")
    wqk = nc.dram_tensor("wqk", [C, 4 * HD], mdt, kind="ExternalInput")
    wv = nc.dram_tensor("wv", [C, CL], mdt, kind="ExternalInput")
    wproj = nc.dram_tensor("wproj", [C, C], mdt, kind="ExternalInput")
    cosd = nc.dram_tensor("cos", [128, T], bf16, kind="ExternalInput")
    sind = nc.dram_tensor("sin", [128, T], bf16, kind="ExternalInput")  # [-s; +s]
    seld = nc.dram_tensor("sel", [NJ, NJ * 128], bf16, kind="ExternalInput")
    out = nc.dram_tensor("out", [B, TOK, C], f32, kind="ExternalOutput")

    # per-(batch, head) AllToAll re-shard buffers: slot d carries this core's
    # head-block yT for tokens [TOK*d, TOK*(d+1)) of batch b.
    a2a_in = [
        [nc.dram_tensor(f"a2a_in{b}_{h}", [N_CORES, HD, TOK], mdt) for h in range(HL)]
        for b in range(B)
    ]
    a2a_out = [
        [nc.dram_tensor(f"a2a_out{b}_{h}", [N_CORES, HD, TOK], mdt) for h in range(HL)]
        for b in range(B)
    ]

    wqk_t = wqk[:, :].rearrange("(ko p) m -> p ko m", p=128)  # [128, KC, 512]
    wv_t = wv[:, :].rearrange("(ko p) m -> p ko m", p=128)  # [128, KC, 256]
    wproj_t = wproj[:, :].rearrange("(ko p) n -> p ko n", p=128)  # [128, KC, 2048]
    # proj lhsT k-tiles: global y-col block k = 2s+h lives in a2a_out[b][h]
    # slot s; view each as [p, s, t].
    yt_t = [
        [a2a_out[b][h][:, :, :].rearrange("s p t -> p s t", p=128) for h in range(HL)]
        for b in range(B)
    ]

    with tile.TileContext(nc) as tc:
        with ExitStack() as top:
            const = top.enter_context(tc.tile_pool(name="const", bufs=1))
            wpool = top.enter_context(tc.tile_pool(name="weights", bufs=1))

            # --- weights first (first matmul waits only on wqk half 0) ---
            wqk_sb = wpool.tile([128, KC, 4 * HD], mdt, tag="wqk")
            for k in range(0, KC, 8):
                nc.sync.dma_start(wqk_sb[:, k : k + 8, :], wqk_t[:, k : k + 8, :])

            with ExitStack() as mid:
                qk_pool = mid.enter_context(tc.tile_pool(name="qkT", bufs=2))
                v_pool = mid.enter_context(tc.tile_pool(name="vsb", bufs=2))
                xk_pool = mid.enter_context(tc.tile_pool(name="xk", bufs=2))
                rtmp = mid.enter_context(tc.tile_pool(name="rtmp", bufs=2))
                apool = mid.enter_context(tc.tile_pool(name="apool", bufs=5))
                ppool = mid.enter_context(tc.tile_pool(name="pair", bufs=2))
                ypool = mid.enter_context(tc.tile_pool(name="yraw", bufs=1))
                spool = mid.enter_context(tc.tile_pool(name="spool", bufs=2))
                spool1 = mid.enter_context(tc.tile_pool(name="spool1", bufs=1))
                wp_pool = mid.enter_context(tc.tile_pool(name="wpr", bufs=2))
                ypj_pool = mid.enter_context(tc.tile_pool(name="ypj", bufs=1))
                opool = mid.enter_context(tc.tile_pool(name="osb", bufs=2))
                # PSUM budget is 8 banks: tags are shared so each pool is
                # bufs x one [128,TQ] f32 bank.
                ps2 = mid.enter_context(tc.tile_pool(name="ps2", bufs=3, space="PSUM"))
                ps_den = mid.enter_context(
                    tc.tile_pool(name="psden", bufs=1, space="PSUM")
                )
                ps_y = mid.enter_context(tc.tile_pool(name="psy", bufs=2, space="PSUM"))
                ps_o = mid.enter_context(tc.tile_pool(name="pso", bufs=2, space="PSUM"))

                # first x chunk prefetch ahead of the constant loads so the
                # first matmul chain starts as early as possible
                xk0 = xk_pool.tile([128, KC, TQ], mdt, tag="xk")
                xv0 = xT[0].rearrange("(ko p) t -> p ko t", p=128)
                nc.sync.dma_start(xk0[:, 0:2, :], xv0[:, 0:2, 0:TQ])
                nc.sync.dma_start(xk0[:, 2:KC, :], xv0[:, 2:KC, 0:TQ])

                # --- constants (after the first-chunk critical path) ---
                ones_f = const.tile([128, 1], f32, tag="ones_f")
                nc.vector.memset(ones_f[:], 1.0)
                ones_b = const.tile([128, 1], mdt, tag="ones_b")
                nc.vector.tensor_copy(ones_b[:], ones_f[:])
                cos_sb = const.tile([128, T], bf16, tag="cos")
                nc.sync.dma_start(cos_sb[:], cosd[:, :])
                sin_sb = const.tile([128, T], bf16, tag="sin")
                nc.sync.dma_start(sin_sb[:], sind[:, :])
                # selector: sel[:, j*128:(j+1)*128] has row j all-ones -> matmul
                # broadcasts rec[j] to 128 partitions.
                sel = const.tile([NJ, NJ * 128], bf16, tag="sel")
                nc.sync.dma_start(sel[:], seld[:, :])

                wv_sb = wpool.tile([128, KC, CL], mdt, tag="wv")
                nc.sync.dma_start(wv_sb[:], wv_t)


                def xk_load(b, n):
                    xv = xT[b].rearrange("(ko p) t -> p ko t", p=128)
                    xk = xk_pool.tile([128, KC, TQ], mdt, tag="xk")
                    nc.sync.dma_start(xk[:], xv[:, :, TQ * n : TQ * (n + 1)])
                    return xk

                def qkv_chunk(b, n, qkT, v_sb, xk=None):
                    """One TQ-column chunk of batch b's QKV projection."""
                    if xk is None:
                        xk = xk_load(b, n)
                    for m in range(4):
                        qk_ps = ps2.tile([128, TQ], f32, tag="mm512")
                        for k in range(KC):
                            nc.tensor.matmul(
                                qk_ps[:],
                                wqk_sb[:, k, 128 * m : 128 * (m + 1)],
                                xk[:, k, :],
                                start=(k == 0),
                                stop=(k == KC - 1),
                            )
                        # rope on the PSUM drain: qkT = ps*cos + swap(ps)*(+-sin)
                        cos_t = cos_sb[:, TQ * n : TQ * (n + 1)]
                        sin_t = sin_sb[:, TQ * n : TQ * (n + 1)]
                        tcs = rtmp.tile([128, TQ], f32, tag="tc")
                        tsn = rtmp.tile([128, TQ], f32, tag="ts")
                        nc.vector.tensor_mul(tcs[:], qk_ps[:], cos_t)
                        nc.vector.tensor_mul(
                            tsn[0:64, :], qk_ps[64:128, :], sin_t[0:64, :]
                        )
                        nc.vector.tensor_mul(
                            tsn[64:128, :], qk_ps[0:64, :], sin_t[64:128, :]
                        )
                        nc.gpsimd.tensor_add(
                            qkT[:, m, TQ * n : TQ * (n + 1)], tcs[:], tsn[:]
                        )
                    for m2 in range(4):
                        v_ps = ps_o.tile([128, TQ], f32, tag="obank")
                        for k in range(KC):
                            nc.tensor.matmul(
                                v_ps[:, 0:CL],
                                xk[:, k, 128 * m2 : 128 * (m2 + 1)],
                                wv_sb[:, k, :],
                                start=(k == 0),
                                stop=(k == KC - 1),
                            )
                        nc.scalar.copy(v_sb[:, 4 * n + m2, :], v_ps[:, 0:CL])

                def attn_chains(b, h, qkT, v_sb, den_all, yraw, js):
                    """Score/attn@v/den chains for head-block h of batch b."""
                    for j in js:
                        idx = NJ * h + j
                        ntk = 4 * j + 4  # causal: k-tiles 0..4j+3
                        yT_ps = ps_y.tile([128, TQ], f32, tag="yT")
                        den_ps = ps_den.tile([1, TQ], f32, tag="den")
                        a_tiles = [None] * ntk
                        nden = 0
                        for i in range(ntk):
                            d = i - 4 * j
                            off = 128 * d if d > 0 else 0
                            sT_ps = ps2.tile([128, TQ], f32, tag="mm512")
                            nc.tensor.matmul(
                                sT_ps[:, off:],
                                qkT[:, 2 + h, 128 * i : 128 * (i + 1)],
                                qkT[:, h, TQ * j + off : TQ * (j + 1)],
                                start=True,
                                stop=True,
                            )
                            a_sb = apool.tile([128, TQ], mdt, tag="a")
                            nc.scalar.activation(a_sb[:, off:], sT_ps[:, off:], Exp)
                            if d >= 0:
                                # causal: zero where tq_local < tk
                                nc.gpsimd.affine_select(
                                    out=a_sb[:, off:],
                                    in_=a_sb[:, off:],
                                    compare_op=mybir.AluOpType.is_ge,
                                    fill=0.0,
                                    base=0,
                                    pattern=[[1, TQ - off]],
                                    channel_multiplier=-1,
                                )
                            a_tiles[i] = a_sb
                            nc.tensor.matmul(
                                yT_ps[:, off:],
                                v_sb[:, i, 128 * h : 128 * (h + 1)],
                                a_sb[:, off:],
                                start=(i == 0),
                                stop=(i == ntk - 1),
                            )
                            # denominator: pair-compact full tiles on DVE,
                            # then one ones-matmul per pair; diagonal tiles
                            # go straight to a sliced ones-matmul.
                            if d < 0 and i % 2 == 1:
                                pr = ppool.tile([128, TQ], mdt, tag="pair")
                                nc.vector.tensor_add(
                                    pr[:], a_tiles[i - 1][:], a_sb[:]
                                )
                                nc.tensor.matmul(
                                    den_ps[:],
                                    ones_b[:],
                                    pr[:],
                                    start=(nden == 0),
                                    stop=False,
                                )
                                nden += 1
                            elif d >= 0:
                                nc.tensor.matmul(
                                    den_ps[:, off:],
                                    ones_b[:],
                                    a_sb[:, off:],
                                    start=(nden == 0),
                                    stop=(d == 3),
                                )
                                nden += 1
                        dtmp = spool.tile([1, TQ], f32, tag="dtmp")
                        nc.vector.tensor_copy(dtmp[:], den_ps[:])
                        nc.sync.dma_start(den_all[j : j + 1, :], dtmp[:])
                        yr = ypool.tile(
                            [128, TQ], f32, tag=f"yraw{idx}", name=f"yr{idx}"
                        )
                        nc.vector.tensor_copy(yr[:], yT_ps[:])
                        yraw[j] = yr

                def attn_finalize(b, h, den_all, yraw):
                    """Reciprocal, normalize, stage + launch the (b,h) A2A."""
                    rec_h = spool1.tile([NJ, TQ], mdt, tag=f"rec{h}")
                    with nc.allow_low_precision(reason="softmax denom recip"):
                        nc.vector.reciprocal(rec_h[:], den_all[:])
                    for j in range(NJ):
                        bc_ps = ps_o.tile([128, TQ], f32, tag="obank")
                        nc.tensor.matmul(
                            bc_ps[:],
                            sel[:, j * 128 : (j + 1) * 128],
                            rec_h[:],
                            start=True,
                            stop=True,
                        )
                        yT_sb = spool.tile([128, TQ], mdt, tag="yT_sb")
                        nc.vector.tensor_mul(yT_sb[:], yraw[j][:], bc_ps[:])
                        nc.sync.dma_start(a2a_in[b][h][2 * j, :, :], yT_sb[:, 0:TOK])
                        nc.sync.dma_start(
                            a2a_in[b][h][2 * j + 1, :, :], yT_sb[:, TOK:TQ]
                        )
                    nc.gpsimd.collective_compute(
                        "AllToAll",
                        mybir.AluOpType.bypass,
                        replica_groups=[list(range(N_CORES))],
                        ins=[a2a_in[b][h][:, :, :]],
                        outs=[a2a_out[b][h][:, :, :]],
                    )

                def proj_head(b):
                    y_sb = ypj_pool.tile([128, KC, TOK], mdt, tag="ypj")
                    for h in range(HL):
                        nc.sync.dma_start(y_sb[:, h : KC : HL, :], yt_t[b][h])
                    return y_sb

                def proj_chunk(b, n, y_sb):
                    wp_sb = wp_pool.tile([128, KC, TQ], mdt, tag="wp")
                    nc.sync.dma_start(wp_sb[:], wproj_t[:, :, TQ * n : TQ * (n + 1)])
                    for m in range(TOK // 128):
                        o_ps = ps_o.tile([128, TQ], f32, tag="obank")
                        for k in range(KC):
                            nc.tensor.matmul(
                                o_ps[:],
                                y_sb[:, k, 128 * m : 128 * (m + 1)],
                                wp_sb[:, k, :],
                                start=(k == 0),
                                stop=(k == KC - 1),
                            )
                        o_sb = opool.tile([128, TQ], f32, tag="o_sb")
                        nc.scalar.copy(o_sb[:], o_ps[:])
                        nc.sync.dma_start(
                            out[b, 128 * m : 128 * (m + 1), TQ * n : TQ * (n + 1)],
                            o_sb[:],
                        )

                # Interleaved schedule: QKV(b+1) / proj chunks between
                # attention j-chain segments, so the PE always has dense
                # independent work while ACT drains the exp pipeline and the
                # A2A transfers fly.
                def qkv_alloc():
                    qkT = qk_pool.tile([128, 4, T], mdt, tag="qkT")
                    v_sb = v_pool.tile([128, NKT, CL], mdt, tag="v")
                    return qkT, v_sb

                cur = qkv_alloc()
                qkv_chunk(0, 0, *cur, xk=xk0)
                for n in range(1, NJ):
                    qkv_chunk(0, n, *cur)
                for b in range(B):
                    nxt = qkv_alloc() if b + 1 < B else None
                    y1 = proj_head(1) if b == B - 1 else None

                    def cover(seg):
                        if nxt is not None:
                            qkv_chunk(b + 1, seg, *nxt)
                        else:
                            proj_chunk(1, seg, y1)

                    qkT, v_sb = cur
                    den0 = spool1.tile([NJ, TQ], f32, tag="den_all0")
                    den1 = spool1.tile([NJ, TQ], f32, tag="den_all1")
                    yr0 = [None] * NJ
                    yr1 = [None] * NJ
                    attn_chains(b, 0, qkT, v_sb, den0, yr0, range(2))
                    cover(0)
                    attn_chains(b, 0, qkT, v_sb, den0, yr0, range(2, NJ))
                    cover(1)
                    attn_chains(b, 1, qkT, v_sb, den1, yr1, range(2))
                    attn_finalize(b, 0, den0, yr0)
                    cover(2)
                    attn_chains(b, 1, qkT, v_sb, den1, yr1, range(2, NJ))
                    if nxt is None:
                        # last batch: launch the final A2A before the last
                        # cover so its transfer hides under proj work
                        attn_finalize(b, 1, den1, yr1)
                        cover(3)
                    else:
                        cover(3)
                        attn_finalize(b, 1, den1, yr1)
                    if b == 1:
                        y_sb = proj_head(0)
                        for n in range(NJ):
                            proj_chunk(0, n, y_sb)
                    cur = nxt
                # tail: proj(2) covers the last A2A transfer, then proj(3)
                # runs on the arrived re-shard.
                y_sb = proj_head(B - 2)
                for n in range(NJ):
                    proj_chunk(B - 2, n, y_sb)
                y_sb = proj_head(B - 1)
                for n in range(NJ):
                    proj_chunk(B - 1, n, y_sb)

    nc.compile()
    return nc


_PERM = None


def _prep_inputs(x, rope, Wqkv, Wproj):
    """Host-side sharding/layout prep (numpy only)."""
    global _PERM
    if _PERM is None:
        _PERM = np.concatenate([np.arange(0, HD, 2), np.arange(1, HD, 2)])
    perm = _PERM

    import ml_dtypes

    mdt_np = ml_dtypes.bfloat16

    x = np.asarray(x, dtype=np.float32)
    xT = np.ascontiguousarray(x.transpose(0, 2, 1)).astype(mdt_np)  # [B, C, T]

    rope = np.asarray(rope, dtype=np.float32)
    cos = np.ascontiguousarray(rope[:, :, 0].T)  # [64, T]
    sin = np.ascontiguousarray(rope[:, :, 1].T)
    coscat = np.concatenate([cos, cos], axis=0).astype(mdt_np)  # [128, T]
    sinpm = np.concatenate([-sin, sin], axis=0).astype(mdt_np)  # [128, T]

    Wqkv = np.asarray(Wqkv, dtype=np.float32)
    Wq = Wqkv[:, 0:C]
    Wk = Wqkv[:, C : 2 * C]
    Wv = Wqkv[:, 2 * C : 3 * C]
    scale = 1.0 / np.sqrt(HD)
    Wproj_m = np.ascontiguousarray(np.asarray(Wproj, dtype=np.float32)).astype(mdt_np)

    sel_np = np.zeros((NJ, NJ * 128), dtype=mdt_np)
    for j in range(NJ):
        sel_np[j, j * 128 : (j + 1) * 128] = 1.0

    in_maps = []
    for c in range(N_CORES):
        cols = []
        for lh in range(HL):
            h = HL * c + lh
            cols.append(h * HD + perm)
        qcols = np.concatenate(cols)
        wq_c = Wq[:, qcols] * scale
        wk_c = Wk[:, qcols]
        wqk_c = np.ascontiguousarray(
            np.concatenate([wq_c, wk_c], axis=1)
        ).astype(mdt_np)  # [C, 512]
        wv_c = np.ascontiguousarray(
            Wv[:, HL * HD * c : HL * HD * (c + 1)]
        ).astype(mdt_np)  # [C, 256]
        in_maps.append(
            {
                "xT": xT,
                "wqk": wqk_c,
                "wv": wv_c,
                "wproj": Wproj_m,
                "cos": coscat,
                "sin": sinpm,
                "sel": sel_np,
            }
        )
    return in_maps


_NC_CACHE = None


def _get_nc():
    global _NC_CACHE
    if _NC_CACHE is None:
        _NC_CACHE = build_program()
    return _NC_CACHE


def run(x, rope, Wqkv, Wproj, trace=False):
    _install_ntff_shim()
    from concourse.bass_utils import run_bass_kernel_spmd

    nc = _get_nc()
    in_maps = _prep_inputs(x, rope, Wqkv, Wproj)
    res = run_bass_kernel_spmd(nc, in_maps, list(range(N_CORES)), trace=trace)
    full = np.empty((B, T, C), dtype=np.float32)
    for c in range(N_CORES):
        full[:, TOK * c : TOK * (c + 1), :] = res.results[c]["out"]
    return full, res


def kernel(x, rope, Wqkv, Wproj):
    out, _ = run(x, rope, Wqkv, Wproj, trace=False)
    return out


if __name__ == "__main__":
    import time

    t0 = time.time()
    nc = build_program()
    ni = sum(len(bb.instructions) for f in nc.m.functions for bb in f.blocks)
    print(f"build ok: {time.time()-t0:.1f}s, {ni} instructions")
